# revision 33
# baseline (speedup 1.0000x reference)
"""Sharded kNN (ArgDistanceMeasure) on 8 TRN2 NeuronCores.

Strategy (FAISS-style sharded kNN):
  - b (the database, [65536, 512]) is sharded row-wise across 8 cores
    (8192 rows each); a (queries, [2048, 512]) is replicated.
  - Ranking identity: argmin_j ||a_i - b_j + eps||^2 over j only needs the
    column-dependent part  score[i,j] = 2*a_i.b_j - c_j  (maximized), where
    c_j = ||b_j||^2 - 2*eps*sum(b_j).  Row-constant terms don't affect
    per-row ranking.
  - Device: per [128 queries x 2048 cols] chunk, a pure fp8-e4m3 DoubleRow
    GEMM (K=256/matmul, 8 matmuls of N=512 at the 157 TF/s fp8 roofline —
    216ns/matmul with LDWEIGHTS hidden behind the in-flight matmul)
    accumulates 2*cross into f32 PSUM split in two 2-bank tiles: ACT copies
    psA (cols 0..1023) to fp16 SBUF while a single DVE tensor_max pairs
    each psB element with its copied mate (TT may read at most one PSUM
    input), and the 1024 fp16 pair-maxima per chunk are DMA'd to HBM.
    Splitting psA/psB lets ACT and DVE free their banks independently, so
    the steady-state chunk period is the PE's 1727ns, not the serialized
    MM->ACT->DVE chain.  Input/output DMAs are spread over the three
    DMA-capable queues (sync, scalar, gpsimd) — one queue sustains only
    ~135 GB/s.
  - Columns of each 2048-wide chunk are host-permuted so that the two
    reduction mates of device pair p are the columns with c-sorted ranks
    2p and 2p+1 (c-adjacent), so the host can subtract a per-pair mean-c
    bias from the pair maxima with negligible approximation error.
  - Host: subtract pair bias, take top-K pairs per (core, seg, query),
    expand each to its 2 columns, recompute the exact fp32 reference
    distance for the ~3k candidates/query, pick the final top-n with the
    reference's tie-break, and apply the reference's buggy index
    bookkeeping.
"""
import numpy as np

NA, D, NB = 2048, 512, 65536
NCORES = 8
NB_SHARD = NB // NCORES  # 8192
CHUNK = 2048             # chunk width (4 PSUM banks)
TOPK_PAIRS = 48          # host keeps top-K pairs per (core, seg, query)
EPS = 1e-6


def build_kernel(na=NA, nb_shard=NB_SHARD, chunk=CHUNK):
    import concourse.mybir as mybir
    from concourse.tile import TileContext

    FP8 = mybir.dt.float8e4
    F16 = mybir.dt.float16
    F32 = mybir.dt.float32
    DR = mybir.MatmulPerfMode.DoubleRow

    nseg = nb_shard // chunk
    nsub = chunk // 512
    half = chunk // 2
    kt = D // 128            # 4 K-tiles of 128
    kp_n = kt // 2           # 2 DoubleRow pairs (K=256 each)
    mt = na // 128

    from concourse import bacc

    nc = bacc.Bacc()

    # DoubleRow operands are [128, 2, cols] (two K-slices packed per
    # partition).  bT is split per chunk group g and K-pair kp so the PE can
    # start on chunk group 0 long before the whole database loads.
    # Chunk group 0 is split into per-j 512-column slices so the first
    # matmul only waits on a 128KB DMA instead of the whole 512KB tile.
    bt0_p = [
        [
            nc.declare_dram_parameter(
                f"bt0p{kp}j{jj}", [128, 2 * 512], FP8, isOutput=False
            )
            for jj in range(nsub)
        ]
        for kp in range(kp_n)
    ]
    bt_p = [
        [
            nc.declare_dram_parameter(
                f"bt{g}p{kp}", [128, 2 * chunk], FP8, isOutput=False
            )
            for kp in range(kp_n)
        ]
        for g in range(1, nseg)
    ]
    at_p = [
        nc.declare_dram_parameter(f"atp{kp}", [128, 2 * na], FP8, isOutput=False)
        for kp in range(kp_n)
    ]
    # First-wave slivers: m=0 chunks need only the first 128 query columns of
    # each at k-pair; loading those 64KB first lets the PE start sooner.
    atpa_p = [
        nc.declare_dram_parameter(f"atp{kp}a", [128, 2 * 128], FP8, isOutput=False)
        for kp in range(kp_n)
    ]
    out_l1 = nc.declare_dram_parameter("out_l1", [na, nseg * half], F16, isOutput=True)

    with TileContext(nc) as tc:
        with (
            tc.tile_pool(name="weights", bufs=1) as wpool,
            tc.tile_pool(name="psumA", bufs=2, space="PSUM") as ppoolA,
            tc.tile_pool(name="psumB", bufs=2, space="PSUM") as ppoolB,
            tc.tile_pool(name="acp", bufs=4) as apool,
            tc.tile_pool(name="l1", bufs=8) as lpool,
        ):
            # Critical ramp set spread over the three DMA-capable engines
            # (sync/SP, scalar/Activation, gpsimd) = three hardware DMA
            # queues: a single queue sustains only ~135 GB/s, which alone
            # made the old 2.1MB critical wave a ~13us ramp.  Queue FIFO
            # order (criticals enqueued first) replaces explicit dep gating.
            # sync: atpa0, bt0p0j0, atpa1, bt0p0j1..j3 — the exact order the
            # first chunk's kp0 matmuls consume; scalar: bt0p1 slices (kp1
            # matmuls start 4 MMs in) then the later chunk groups; gpsimd:
            # both full at tensors (kp0 first — m=1 needs it ~0.9us sooner).
            bt0_t = [[None] * nsub for _ in range(kp_n)]
            atpa = []
            t = wpool.tile([128, 2 * 128], FP8, tag="atp0a", name="atp0a")
            nc.sync.dma_start(out=t, in_=atpa_p[0][:, :])
            atpa.append(t)
            t = wpool.tile([128, 2 * 512], FP8, tag="bt0p0j0", name="bt0p0j0")
            nc.sync.dma_start(out=t, in_=bt0_p[0][0][:, :])
            bt0_t[0][0] = t
            t = wpool.tile([128, 2 * 128], FP8, tag="atp1a", name="atp1a")
            nc.sync.dma_start(out=t, in_=atpa_p[1][:, :])
            atpa.append(t)
            for jj in range(1, nsub):
                t = wpool.tile(
                    [128, 2 * 512], FP8, tag=f"bt0p0j{jj}", name=f"bt0p0j{jj}"
                )
                nc.sync.dma_start(out=t, in_=bt0_p[0][jj][:, :])
                bt0_t[0][jj] = t
            for jj in range(nsub):
                t = wpool.tile(
                    [128, 2 * 512], FP8, tag=f"bt0p1j{jj}", name=f"bt0p1j{jj}"
                )
                nc.scalar.dma_start(out=t, in_=bt0_p[1][jj][:, :])
                bt0_t[1][jj] = t
            at_t = []
            for kp in range(kp_n):
                t = wpool.tile([128, 2 * na], FP8, tag=f"atp{kp}", name=f"atp{kp}")
                nc.gpsimd.dma_start(out=t, in_=at_p[kp][:, :])
                at_t.append(t)
            # Non-critical preloads ride the scalar queue behind bt0's slices.
            bt_t = [[None] * kp_n for _ in range(nseg - 1)]
            for g in range(nseg - 1):
                for kp in range(kp_n):
                    t = wpool.tile(
                        [128, 2 * chunk], FP8, tag=f"bt{g + 1}p{kp}", name=f"bt{g + 1}p{kp}"
                    )
                    nc.scalar.dma_start(out=t, in_=bt_p[g][kp][:, :])
                    bt_t[g][kp] = t
            atpa3 = [t.rearrange("p (i c) -> p i c", i=2) for t in atpa]

            at3 = [t.rearrange("p (i c) -> p i c", i=2) for t in at_t]
            bt03 = [
                [t.rearrange("p (i c) -> p i c", i=2) for t in row] for row in bt0_t
            ]
            bt3 = [
                [t.rearrange("p (i c) -> p i c", i=2) for t in row] for row in bt_t
            ]

            for s in range(nseg):
                for m in range(mt):
                    # Split PSUM per chunk: psA (cols 0..1023, consumed by
                    # ACT) and psB (cols 1024..2047, consumed by DVE) free
                    # independently, so the MM->ACT->DVE chain no longer
                    # bounds the period at (MM+ACT+DVE)/2 — the PE does.
                    psA = ppoolA.tile([128, half], F32, tag="psA")
                    psB = ppoolB.tile([128, half], F32, tag="psB")
                    for kp in range(kp_n):
                        for j in range(nsub):
                            if m == 0:
                                lhsT3 = atpa3[kp][:, :, :]
                            else:
                                lhsT3 = at3[kp][:, :, m * 128 : (m + 1) * 128]
                            if s == 0:
                                rhs3 = bt03[kp][j][:, :, :]
                            else:
                                rhs3 = bt3[s - 1][kp][:, :, j * 512 : (j + 1) * 512]
                            if j < nsub // 2:
                                dst = psA[:, j * 512 : (j + 1) * 512]
                            else:
                                dst = psB[:, (j - nsub // 2) * 512 : (j - nsub // 2 + 1) * 512]
                            nc.tensor.matmul(
                                dst,
                                lhsT3,
                                rhs3,
                                start=(kp == 0),
                                stop=(kp == kp_n - 1),
                                perf_mode=DR,
                            )
                    # ACT converts the front half of PSUM to fp16 SBUF; the
                    # single DVE op pairs each back-half PSUM element with its
                    # copied mate (TT may read at most ONE input from PSUM).
                    ca = apool.tile([128, half], F16, tag="ca")
                    nc.scalar.copy(out=ca, in_=psA)
                    l1 = lpool.tile([128, half], F16, tag="l1")
                    nc.vector.tensor_max(l1, psB, ca)
                    # Output DMAs alternate between the gpsimd and sync
                    # queues — one queue alone (~135 GB/s) would need
                    # ~1.9us/chunk for the 256KB l1 tile, above the PE
                    # period.
                    oeng = nc.gpsimd if (s * mt + m) % 2 == 0 else nc.sync
                    oeng.dma_start(
                        out=out_l1[m * 128 : (m + 1) * 128, s * half : (s + 1) * half],
                        in_=l1,
                    )
    nc.compile()
    return nc


def make_in_maps(a, b):
    """Pack per-core inputs.  Columns of each 2048-wide chunk are permuted so
    that the two level-1 reduction mates of device pair p hold the columns
    with c-sorted ranks 2p and 2p+1.  Device pairing: l1[p] = max of
    positions (p, 1024+p) [position p rides the ACT fp16 copy, 1024+p is
    read raw from PSUM], so position q maps to rank 2q for q<1024 and
    2(q-1024)+1 for q>=1024.
    Returns (in_maps, ranks) where ranks[core][s][r] is the local column with
    the r-th smallest c."""
    import ml_dtypes

    kt = D // 128
    kp_n = kt // 2
    aT8 = (2.0 * a).T.astype(ml_dtypes.float8_e4m3)   # [512, NA]
    bT8 = b.T.astype(ml_dtypes.float8_e4m3)           # [512, NB]
    b2 = np.einsum("ij,ij->i", b, b)
    sb = b.sum(axis=1)
    c = (b2 - np.float32(2.0 * EPS) * sb).astype(np.float32)
    nseg = NB_SHARD // CHUNK
    half = CHUNK // 2

    # device position q -> c-sorted rank: pair p = (p, 1024+p) -> (2p, 2p+1)
    q = np.arange(CHUNK)
    r_of_q = np.where(q < half, 2 * q, 2 * (q - half) + 1)

    def pair_pack(mat, kp):
        # [128, 2*cols]: slot i holds K-tile (kp*2+i) rows of mat
        lo = mat[(kp * 2) * 128 : (kp * 2 + 1) * 128, :]
        hi = mat[(kp * 2 + 1) * 128 : (kp * 2 + 2) * 128, :]
        return np.ascontiguousarray(np.concatenate([lo, hi], axis=1))

    in_maps = []
    all_ranks = []
    for core in range(NCORES):
        sl = slice(core * NB_SHARD, (core + 1) * NB_SHARD)
        bT = bT8[:, sl]
        c_core = c[core * NB_SHARD : (core + 1) * NB_SHARD]
        ranks = []
        im = {}
        for kp in range(kp_n):
            im[f"atp{kp}"] = pair_pack(aT8, kp)
            im[f"atp{kp}a"] = pair_pack(aT8[:, 0:128], kp)
        for g in range(nseg):
            cch = c_core[g * CHUNK : (g + 1) * CHUNK]
            rank = np.argsort(cch, kind="stable")
            ranks.append(rank)
            perm = rank[r_of_q]
            cols = np.ascontiguousarray(bT[:, g * CHUNK : (g + 1) * CHUNK][:, perm])
            if g == 0:
                for kp in range(kp_n):
                    for jj in range(CHUNK // 512):
                        im[f"bt0p{kp}j{jj}"] = pair_pack(
                            np.ascontiguousarray(cols[:, jj * 512 : (jj + 1) * 512]),
                            kp,
                        )
            else:
                for kp in range(kp_n):
                    im[f"bt{g}p{kp}"] = pair_pack(cols, kp)
        in_maps.append(im)
        all_ranks.append(ranks)
    return in_maps, all_ranks


def merge_results(a, b, n, b_batch_size, results, all_ranks):
    """Subtract the per-pair mean-c bias from the device pair-maxima, take
    the top-K pairs per (core, seg, query), expand each winner to its 2
    c-adjacent columns, refine with the exact fp32 reference distance, pick
    final top-n (ties -> lowest index), apply the reference's buggy
    bookkeeping.  Device pair p of (core, seg) covers the columns with
    c-ranks 2p and 2p+1 by construction of r_of_q."""
    nseg = NB_SHARD // CHUNK
    half = CHUNK // 2
    na = a.shape[0]
    b2 = np.sum(b * b, axis=1)
    sb_ = np.sum(b, axis=1)
    c = (b2 - np.float32(2.0 * EPS) * sb_).astype(np.float32)

    K = TOPK_PAIRS
    cand_parts = []
    for core in range(NCORES):
        l1 = results[core]["out_l1"].astype(np.float32)  # [NA, nseg*half]
        base_core = core * NB_SHARD
        for s in range(nseg):
            rank = all_ranks[core][s]
            # columns of pair p: ranks 2p, 2p+1
            cols0 = rank[0::2]  # [half]
            cols1 = rank[1::2]
            cpair = 0.5 * (c[base_core + s * CHUNK + cols0]
                           + c[base_core + s * CHUNK + cols1])
            corr = l1[:, s * half : (s + 1) * half] - cpair[None, :]
            top = np.argpartition(-corr, K, axis=1)[:, :K]  # [NA, K]
            base = base_core + s * CHUNK
            cand_parts.append(cols0[top] + base)
            cand_parts.append(cols1[top] + base)
    cand = np.concatenate(cand_parts, axis=1)  # [NA, NCORES*nseg*2K]

    a2 = np.sum(a * a, axis=1)
    sa = np.sum(a, axis=1)
    d = a.shape[1]
    out = np.empty((na, n), dtype=np.int64)
    CHQ = 128
    eps = np.float32(EPS)
    for q0 in range(0, na, CHQ):
        q1 = min(q0 + CHQ, na)
        Cc = cand[q0:q1]
        Bc = b[Cc]
        cross = np.matmul(Bc, a[q0:q1, :, None])[..., 0].astype(np.float32)
        sq = (
            a2[q0:q1, None]
            + b2[Cc]
            - np.float32(2.0) * cross
            + np.float32(2.0) * eps * (sa[q0:q1, None] - sb_[Cc])
            + np.float32(d) * eps * eps
        )
        dist = np.sqrt(np.maximum(sq, np.float32(0.0)))
        ordr = np.lexsort((Cc, dist), axis=1)[:, :n]
        rows = np.arange(q1 - q0)[:, None]
        out[q0:q1] = Cc[rows, ordr]
    buggy = (out % b_batch_size) + (out // b_batch_size)
    return buggy.astype(np.int64)


def kernel(a, b, n, b_batch_size, trace=False):
    from concourse.bass_utils import run_bass_kernel_spmd

    a = np.ascontiguousarray(np.asarray(a, dtype=np.float32))
    b = np.ascontiguousarray(np.asarray(b, dtype=np.float32))
    n = int(n)
    b_batch_size = int(b_batch_size)

    nc = build_kernel()
    in_maps, all_ranks = make_in_maps(a, b)
    res = run_bass_kernel_spmd(
        nc, in_maps, core_ids=list(range(NCORES)), trace=trace
    )
    out = merge_results(a, b, n, b_batch_size, res.results, all_ranks)
    if trace:
        return out, res
    return out


# revision 34
# speedup vs baseline: 1.0011x; 1.0011x over previous
"""Sharded kNN (ArgDistanceMeasure) on 8 TRN2 NeuronCores.

Strategy (FAISS-style sharded kNN):
  - b (the database, [65536, 512]) is sharded row-wise across 8 cores
    (8192 rows each); a (queries, [2048, 512]) is replicated.
  - Ranking identity: argmin_j ||a_i - b_j + eps||^2 over j only needs the
    column-dependent part  score[i,j] = 2*a_i.b_j - c_j  (maximized), where
    c_j = ||b_j||^2 - 2*eps*sum(b_j).  Row-constant terms don't affect
    per-row ranking.
  - Device: per [128 queries x 2048 cols] chunk, a pure fp8-e4m3 DoubleRow
    GEMM (K=256/matmul, 8 matmuls of N=512 at the 157 TF/s fp8 roofline —
    216ns/matmul with LDWEIGHTS hidden behind the in-flight matmul)
    accumulates 2*cross into f32 PSUM split in two 2-bank tiles: ACT copies
    psA (cols 0..1023) to fp16 SBUF while a single DVE tensor_max pairs
    each psB element with its copied mate (TT may read at most one PSUM
    input), and the 1024 fp16 pair-maxima per chunk are DMA'd to HBM.
    Splitting psA/psB lets ACT and DVE free their banks independently, so
    the steady-state chunk period is the PE's 1727ns, not the serialized
    MM->ACT->DVE chain.  Input/output DMAs are spread over the three
    DMA-capable queues (sync, scalar, gpsimd) — one queue sustains only
    ~135 GB/s.
  - Columns of each 2048-wide chunk are host-permuted so that the two
    reduction mates of device pair p are the columns with c-sorted ranks
    2p and 2p+1 (c-adjacent), so the host can subtract a per-pair mean-c
    bias from the pair maxima with negligible approximation error.
  - Host: subtract pair bias, take top-K pairs per (core, seg, query),
    expand each to its 2 columns, recompute the exact fp32 reference
    distance for the ~3k candidates/query, pick the final top-n with the
    reference's tie-break, and apply the reference's buggy index
    bookkeeping.
"""
import numpy as np

NA, D, NB = 2048, 512, 65536
NCORES = 8
NB_SHARD = NB // NCORES  # 8192
CHUNK = 2048             # chunk width (4 PSUM banks)
TOPK_PAIRS = 48          # host keeps top-K pairs per (core, seg, query)
EPS = 1e-6


def build_kernel(na=NA, nb_shard=NB_SHARD, chunk=CHUNK):
    import concourse.mybir as mybir
    from concourse.tile import TileContext

    FP8 = mybir.dt.float8e4
    F16 = mybir.dt.float16
    F32 = mybir.dt.float32
    DR = mybir.MatmulPerfMode.DoubleRow

    nseg = nb_shard // chunk
    nsub = chunk // 512
    half = chunk // 2
    kt = D // 128            # 4 K-tiles of 128
    kp_n = kt // 2           # 2 DoubleRow pairs (K=256 each)
    mt = na // 128

    from concourse import bacc

    nc = bacc.Bacc()

    # DoubleRow operands are [128, 2, cols] (two K-slices packed per
    # partition).  bT is split per chunk group g and K-pair kp so the PE can
    # start on chunk group 0 long before the whole database loads.
    # Chunk group 0 is split into per-j 512-column slices so the first
    # matmul only waits on a 128KB DMA instead of the whole 512KB tile.
    bt0_p = [
        [
            nc.declare_dram_parameter(
                f"bt0p{kp}j{jj}", [128, 2 * 512], FP8, isOutput=False
            )
            for jj in range(nsub)
        ]
        for kp in range(kp_n)
    ]
    bt_p = [
        [
            nc.declare_dram_parameter(
                f"bt{g}p{kp}", [128, 2 * chunk], FP8, isOutput=False
            )
            for kp in range(kp_n)
        ]
        for g in range(1, nseg)
    ]
    at_p = [
        nc.declare_dram_parameter(f"atp{kp}", [128, 2 * na], FP8, isOutput=False)
        for kp in range(kp_n)
    ]
    # First-wave slivers: m=0 chunks need only the first 128 query columns of
    # each at k-pair; loading those 64KB first lets the PE start sooner.
    atpa_p = [
        nc.declare_dram_parameter(f"atp{kp}a", [128, 2 * 128], FP8, isOutput=False)
        for kp in range(kp_n)
    ]
    out_l1 = nc.declare_dram_parameter("out_l1", [na, nseg * half], F16, isOutput=True)

    with TileContext(nc) as tc:
        with (
            tc.tile_pool(name="weights", bufs=1) as wpool,
            tc.tile_pool(name="psumA", bufs=2, space="PSUM") as ppoolA,
            tc.tile_pool(name="psumB", bufs=2, space="PSUM") as ppoolB,
            tc.tile_pool(name="acp", bufs=4) as apool,
            tc.tile_pool(name="l1", bufs=8) as lpool,
        ):
            # Critical ramp set spread over the three DMA-capable engines
            # (sync/SP, scalar/Activation, gpsimd) = three hardware DMA
            # queues: a single queue sustains only ~135 GB/s, which alone
            # made the old 2.1MB critical wave a ~13us ramp.  Queue FIFO
            # order (criticals enqueued first) replaces explicit dep gating.
            # sync: atpa0, bt0p0j0, atpa1, bt0p0j1..j3 — the exact order the
            # first chunk's kp0 matmuls consume; scalar: bt0p1 slices (kp1
            # matmuls start 4 MMs in) then the later chunk groups; gpsimd:
            # both full at tensors (kp0 first — m=1 needs it ~0.9us sooner).
            bt0_t = [[None] * nsub for _ in range(kp_n)]
            atpa = []
            t = wpool.tile([128, 2 * 128], FP8, tag="atp0a", name="atp0a")
            nc.sync.dma_start(out=t, in_=atpa_p[0][:, :])
            atpa.append(t)
            t = wpool.tile([128, 2 * 512], FP8, tag="bt0p0j0", name="bt0p0j0")
            nc.sync.dma_start(out=t, in_=bt0_p[0][0][:, :])
            bt0_t[0][0] = t
            t = wpool.tile([128, 2 * 128], FP8, tag="atp1a", name="atp1a")
            nc.sync.dma_start(out=t, in_=atpa_p[1][:, :])
            atpa.append(t)
            for jj in range(1, nsub):
                t = wpool.tile(
                    [128, 2 * 512], FP8, tag=f"bt0p0j{jj}", name=f"bt0p0j{jj}"
                )
                nc.sync.dma_start(out=t, in_=bt0_p[0][jj][:, :])
                bt0_t[0][jj] = t
            for jj in range(nsub):
                t = wpool.tile(
                    [128, 2 * 512], FP8, tag=f"bt0p1j{jj}", name=f"bt0p1j{jj}"
                )
                nc.scalar.dma_start(out=t, in_=bt0_p[1][jj][:, :])
                bt0_t[1][jj] = t
            at_t = []
            t = wpool.tile([128, 2 * na], FP8, tag="atp0", name="atp0")
            nc.sync.dma_start(out=t, in_=at_p[0][:, :])
            at_t.append(t)
            t = wpool.tile([128, 2 * na], FP8, tag="atp1", name="atp1")
            nc.gpsimd.dma_start(out=t, in_=at_p[1][:, :])
            at_t.append(t)
            # Non-critical preloads ride the scalar queue behind bt0's slices.
            bt_t = [[None] * kp_n for _ in range(nseg - 1)]
            for g in range(nseg - 1):
                for kp in range(kp_n):
                    t = wpool.tile(
                        [128, 2 * chunk], FP8, tag=f"bt{g + 1}p{kp}", name=f"bt{g + 1}p{kp}"
                    )
                    nc.scalar.dma_start(out=t, in_=bt_p[g][kp][:, :])
                    bt_t[g][kp] = t
            atpa3 = [t.rearrange("p (i c) -> p i c", i=2) for t in atpa]

            at3 = [t.rearrange("p (i c) -> p i c", i=2) for t in at_t]
            bt03 = [
                [t.rearrange("p (i c) -> p i c", i=2) for t in row] for row in bt0_t
            ]
            bt3 = [
                [t.rearrange("p (i c) -> p i c", i=2) for t in row] for row in bt_t
            ]

            for s in range(nseg):
                for m in range(mt):
                    # Split PSUM per chunk: psA (cols 0..1023, consumed by
                    # ACT) and psB (cols 1024..2047, consumed by DVE) free
                    # independently, so the MM->ACT->DVE chain no longer
                    # bounds the period at (MM+ACT+DVE)/2 — the PE does.
                    psA = ppoolA.tile([128, half], F32, tag="psA")
                    psB = ppoolB.tile([128, half], F32, tag="psB")
                    for kp in range(kp_n):
                        for j in range(nsub):
                            if m == 0:
                                lhsT3 = atpa3[kp][:, :, :]
                            else:
                                lhsT3 = at3[kp][:, :, m * 128 : (m + 1) * 128]
                            if s == 0:
                                rhs3 = bt03[kp][j][:, :, :]
                            else:
                                rhs3 = bt3[s - 1][kp][:, :, j * 512 : (j + 1) * 512]
                            if j < nsub // 2:
                                dst = psA[:, j * 512 : (j + 1) * 512]
                            else:
                                dst = psB[:, (j - nsub // 2) * 512 : (j - nsub // 2 + 1) * 512]
                            nc.tensor.matmul(
                                dst,
                                lhsT3,
                                rhs3,
                                start=(kp == 0),
                                stop=(kp == kp_n - 1),
                                perf_mode=DR,
                            )
                    # ACT converts the front half of PSUM to fp16 SBUF; the
                    # single DVE op pairs each back-half PSUM element with its
                    # copied mate (TT may read at most ONE input from PSUM).
                    ca = apool.tile([128, half], F16, tag="ca")
                    nc.scalar.copy(out=ca, in_=psA)
                    l1 = lpool.tile([128, half], F16, tag="l1")
                    nc.vector.tensor_max(l1, psB, ca)
                    # Output DMAs alternate between the gpsimd and sync
                    # queues — one queue alone (~135 GB/s) would need
                    # ~1.9us/chunk for the 256KB l1 tile, above the PE
                    # period.
                    oeng = nc.gpsimd if (s * mt + m) % 2 == 0 else nc.sync
                    oeng.dma_start(
                        out=out_l1[m * 128 : (m + 1) * 128, s * half : (s + 1) * half],
                        in_=l1,
                    )
    nc.compile()
    return nc


def make_in_maps(a, b):
    """Pack per-core inputs.  Columns of each 2048-wide chunk are permuted so
    that the two level-1 reduction mates of device pair p hold the columns
    with c-sorted ranks 2p and 2p+1.  Device pairing: l1[p] = max of
    positions (p, 1024+p) [position p rides the ACT fp16 copy, 1024+p is
    read raw from PSUM], so position q maps to rank 2q for q<1024 and
    2(q-1024)+1 for q>=1024.
    Returns (in_maps, ranks) where ranks[core][s][r] is the local column with
    the r-th smallest c."""
    import ml_dtypes

    kt = D // 128
    kp_n = kt // 2
    aT8 = (2.0 * a).T.astype(ml_dtypes.float8_e4m3)   # [512, NA]
    bT8 = b.T.astype(ml_dtypes.float8_e4m3)           # [512, NB]
    b2 = np.einsum("ij,ij->i", b, b)
    sb = b.sum(axis=1)
    c = (b2 - np.float32(2.0 * EPS) * sb).astype(np.float32)
    nseg = NB_SHARD // CHUNK
    half = CHUNK // 2

    # device position q -> c-sorted rank: pair p = (p, 1024+p) -> (2p, 2p+1)
    q = np.arange(CHUNK)
    r_of_q = np.where(q < half, 2 * q, 2 * (q - half) + 1)

    def pair_pack(mat, kp):
        # [128, 2*cols]: slot i holds K-tile (kp*2+i) rows of mat
        lo = mat[(kp * 2) * 128 : (kp * 2 + 1) * 128, :]
        hi = mat[(kp * 2 + 1) * 128 : (kp * 2 + 2) * 128, :]
        return np.ascontiguousarray(np.concatenate([lo, hi], axis=1))

    in_maps = []
    all_ranks = []
    for core in range(NCORES):
        sl = slice(core * NB_SHARD, (core + 1) * NB_SHARD)
        bT = bT8[:, sl]
        c_core = c[core * NB_SHARD : (core + 1) * NB_SHARD]
        ranks = []
        im = {}
        for kp in range(kp_n):
            im[f"atp{kp}"] = pair_pack(aT8, kp)
            im[f"atp{kp}a"] = pair_pack(aT8[:, 0:128], kp)
        for g in range(nseg):
            cch = c_core[g * CHUNK : (g + 1) * CHUNK]
            rank = np.argsort(cch, kind="stable")
            ranks.append(rank)
            perm = rank[r_of_q]
            cols = np.ascontiguousarray(bT[:, g * CHUNK : (g + 1) * CHUNK][:, perm])
            if g == 0:
                for kp in range(kp_n):
                    for jj in range(CHUNK // 512):
                        im[f"bt0p{kp}j{jj}"] = pair_pack(
                            np.ascontiguousarray(cols[:, jj * 512 : (jj + 1) * 512]),
                            kp,
                        )
            else:
                for kp in range(kp_n):
                    im[f"bt{g}p{kp}"] = pair_pack(cols, kp)
        in_maps.append(im)
        all_ranks.append(ranks)
    return in_maps, all_ranks


def merge_results(a, b, n, b_batch_size, results, all_ranks):
    """Subtract the per-pair mean-c bias from the device pair-maxima, take
    the top-K pairs per (core, seg, query), expand each winner to its 2
    c-adjacent columns, refine with the exact fp32 reference distance, pick
    final top-n (ties -> lowest index), apply the reference's buggy
    bookkeeping.  Device pair p of (core, seg) covers the columns with
    c-ranks 2p and 2p+1 by construction of r_of_q."""
    nseg = NB_SHARD // CHUNK
    half = CHUNK // 2
    na = a.shape[0]
    b2 = np.sum(b * b, axis=1)
    sb_ = np.sum(b, axis=1)
    c = (b2 - np.float32(2.0 * EPS) * sb_).astype(np.float32)

    K = TOPK_PAIRS
    cand_parts = []
    for core in range(NCORES):
        l1 = results[core]["out_l1"].astype(np.float32)  # [NA, nseg*half]
        base_core = core * NB_SHARD
        for s in range(nseg):
            rank = all_ranks[core][s]
            # columns of pair p: ranks 2p, 2p+1
            cols0 = rank[0::2]  # [half]
            cols1 = rank[1::2]
            cpair = 0.5 * (c[base_core + s * CHUNK + cols0]
                           + c[base_core + s * CHUNK + cols1])
            corr = l1[:, s * half : (s + 1) * half] - cpair[None, :]
            top = np.argpartition(-corr, K, axis=1)[:, :K]  # [NA, K]
            base = base_core + s * CHUNK
            cand_parts.append(cols0[top] + base)
            cand_parts.append(cols1[top] + base)
    cand = np.concatenate(cand_parts, axis=1)  # [NA, NCORES*nseg*2K]

    a2 = np.sum(a * a, axis=1)
    sa = np.sum(a, axis=1)
    d = a.shape[1]
    out = np.empty((na, n), dtype=np.int64)
    CHQ = 128
    eps = np.float32(EPS)
    for q0 in range(0, na, CHQ):
        q1 = min(q0 + CHQ, na)
        Cc = cand[q0:q1]
        Bc = b[Cc]
        cross = np.matmul(Bc, a[q0:q1, :, None])[..., 0].astype(np.float32)
        sq = (
            a2[q0:q1, None]
            + b2[Cc]
            - np.float32(2.0) * cross
            + np.float32(2.0) * eps * (sa[q0:q1, None] - sb_[Cc])
            + np.float32(d) * eps * eps
        )
        dist = np.sqrt(np.maximum(sq, np.float32(0.0)))
        ordr = np.lexsort((Cc, dist), axis=1)[:, :n]
        rows = np.arange(q1 - q0)[:, None]
        out[q0:q1] = Cc[rows, ordr]
    buggy = (out % b_batch_size) + (out // b_batch_size)
    return buggy.astype(np.int64)


def kernel(a, b, n, b_batch_size, trace=False):
    from concourse.bass_utils import run_bass_kernel_spmd

    a = np.ascontiguousarray(np.asarray(a, dtype=np.float32))
    b = np.ascontiguousarray(np.asarray(b, dtype=np.float32))
    n = int(n)
    b_batch_size = int(b_batch_size)

    nc = build_kernel()
    in_maps, all_ranks = make_in_maps(a, b)
    res = run_bass_kernel_spmd(
        nc, in_maps, core_ids=list(range(NCORES)), trace=trace
    )
    out = merge_results(a, b, n, b_batch_size, res.results, all_ranks)
    if trace:
        return out, res
    return out


# revision 36
# speedup vs baseline: 1.0021x; 1.0010x over previous
"""Sharded kNN (ArgDistanceMeasure) on 8 TRN2 NeuronCores.

Strategy (FAISS-style sharded kNN):
  - b (the database, [65536, 512]) is sharded row-wise across 8 cores
    (8192 rows each); a (queries, [2048, 512]) is replicated.
  - Ranking identity: argmin_j ||a_i - b_j + eps||^2 over j only needs the
    column-dependent part  score[i,j] = 2*a_i.b_j - c_j  (maximized), where
    c_j = ||b_j||^2 - 2*eps*sum(b_j).  Row-constant terms don't affect
    per-row ranking.
  - Device: per [128 queries x 2048 cols] chunk, a pure fp8-e4m3 DoubleRow
    GEMM (K=256/matmul, 8 matmuls of N=512 at the 157 TF/s fp8 roofline —
    216ns/matmul with LDWEIGHTS hidden behind the in-flight matmul)
    accumulates 2*cross into f32 PSUM split in two 2-bank tiles: ACT copies
    psA (cols 0..1023) to fp16 SBUF while a single DVE tensor_max pairs
    each psB element with its copied mate (TT may read at most one PSUM
    input), and the 1024 fp16 pair-maxima per chunk are DMA'd to HBM.
    Splitting psA/psB lets ACT and DVE free their banks independently, so
    the steady-state chunk period is the PE's 1727ns, not the serialized
    MM->ACT->DVE chain.  Input/output DMAs are spread over the three
    DMA-capable queues (sync, scalar, gpsimd) — one queue sustains only
    ~135 GB/s.
  - Columns of each 2048-wide chunk are host-permuted so that the two
    reduction mates of device pair p are the columns with c-sorted ranks
    2p and 2p+1 (c-adjacent), so the host can subtract a per-pair mean-c
    bias from the pair maxima with negligible approximation error.
  - Host: subtract pair bias, take top-K pairs per (core, seg, query),
    expand each to its 2 columns, recompute the exact fp32 reference
    distance for the ~3k candidates/query, pick the final top-n with the
    reference's tie-break, and apply the reference's buggy index
    bookkeeping.
"""
import numpy as np

NA, D, NB = 2048, 512, 65536
NCORES = 8
NB_SHARD = NB // NCORES  # 8192
CHUNK = 2048             # chunk width (4 PSUM banks)
TOPK_PAIRS = 48          # host keeps top-K pairs per (core, seg, query)
EPS = 1e-6


def build_kernel(na=NA, nb_shard=NB_SHARD, chunk=CHUNK):
    import concourse.mybir as mybir
    from concourse.tile import TileContext

    FP8 = mybir.dt.float8e4
    F16 = mybir.dt.float16
    F32 = mybir.dt.float32
    DR = mybir.MatmulPerfMode.DoubleRow

    nseg = nb_shard // chunk
    nsub = chunk // 512
    half = chunk // 2
    kt = D // 128            # 4 K-tiles of 128
    kp_n = kt // 2           # 2 DoubleRow pairs (K=256 each)
    mt = na // 128

    from concourse import bacc

    nc = bacc.Bacc()

    # DoubleRow operands are [128, 2, cols] (two K-slices packed per
    # partition).  bT is split per chunk group g and K-pair kp so the PE can
    # start on chunk group 0 long before the whole database loads.
    # Chunk group 0 is split into per-j 512-column slices so the first
    # matmul only waits on a 128KB DMA instead of the whole 512KB tile.
    bt0_p = [
        [
            nc.declare_dram_parameter(
                f"bt0p{kp}j{jj}", [128, 2 * 512], FP8, isOutput=False
            )
            for jj in range(nsub)
        ]
        for kp in range(kp_n)
    ]
    bt_p = [
        [
            nc.declare_dram_parameter(
                f"bt{g}p{kp}", [128, 2 * chunk], FP8, isOutput=False
            )
            for kp in range(kp_n)
        ]
        for g in range(1, nseg)
    ]
    at_p = [
        nc.declare_dram_parameter(f"atp{kp}", [128, 2 * na], FP8, isOutput=False)
        for kp in range(kp_n)
    ]
    # First-wave slivers: m=0 chunks need only the first 128 query columns of
    # each at k-pair; loading those 64KB first lets the PE start sooner.
    atpa_p = [
        nc.declare_dram_parameter(f"atp{kp}a", [128, 2 * 128], FP8, isOutput=False)
        for kp in range(kp_n)
    ]
    out_l1 = nc.declare_dram_parameter("out_l1", [na, nseg * half], F16, isOutput=True)

    with TileContext(nc) as tc:
        with (
            tc.tile_pool(name="weights", bufs=1) as wpool,
            tc.tile_pool(name="psumA", bufs=2, space="PSUM") as ppoolA,
            tc.tile_pool(name="psumB", bufs=2, space="PSUM") as ppoolB,
            tc.tile_pool(name="acp", bufs=3) as apool,
            tc.tile_pool(name="l1", bufs=6) as lpool,
        ):
            # Critical ramp set spread over the three DMA-capable engines
            # (sync/SP, scalar/Activation, gpsimd) = three hardware DMA
            # queues: a single queue sustains only ~135 GB/s, which alone
            # made the old 2.1MB critical wave a ~13us ramp.  Queue FIFO
            # order (criticals enqueued first) replaces explicit dep gating.
            atpa = []
            for kp in range(kp_n):
                t = wpool.tile([128, 2 * 128], FP8, tag=f"atp{kp}a", name=f"atp{kp}a")
                nc.sync.dma_start(out=t, in_=atpa_p[kp][:, :])
                atpa.append(t)
            bt0_t = [[None] * nsub for _ in range(kp_n)]
            for jj in range(nsub):
                for kp in range(kp_n):
                    t = wpool.tile(
                        [128, 2 * 512], FP8, tag=f"bt0p{kp}j{jj}", name=f"bt0p{kp}j{jj}"
                    )
                    eng = nc.sync if kp == 0 else nc.scalar
                    eng.dma_start(out=t, in_=bt0_p[kp][jj][:, :])
                    bt0_t[kp][jj] = t
            at_t = []
            t = wpool.tile([128, 2 * na], FP8, tag="atp0", name="atp0")
            nc.sync.dma_start(out=t, in_=at_p[0][:, :])
            at_t.append(t)
            t = wpool.tile([128, 2 * na], FP8, tag="atp1", name="atp1")
            nc.gpsimd.dma_start(out=t, in_=at_p[1][:, :])
            at_t.append(t)
            # Non-critical preloads ride the scalar queue behind bt0's slices.
            bt_t = [[None] * kp_n for _ in range(nseg - 1)]
            for g in range(nseg - 1):
                for kp in range(kp_n):
                    t = wpool.tile(
                        [128, 2 * chunk], FP8, tag=f"bt{g + 1}p{kp}", name=f"bt{g + 1}p{kp}"
                    )
                    nc.scalar.dma_start(out=t, in_=bt_p[g][kp][:, :])
                    bt_t[g][kp] = t
            atpa3 = [t.rearrange("p (i c) -> p i c", i=2) for t in atpa]

            at3 = [t.rearrange("p (i c) -> p i c", i=2) for t in at_t]
            bt03 = [
                [t.rearrange("p (i c) -> p i c", i=2) for t in row] for row in bt0_t
            ]
            bt3 = [
                [t.rearrange("p (i c) -> p i c", i=2) for t in row] for row in bt_t
            ]

            for s in range(nseg):
                for m in range(mt):
                    # Split PSUM per chunk: psA (cols 0..1023, consumed by
                    # ACT) and psB (cols 1024..2047, consumed by DVE) free
                    # independently, so the MM->ACT->DVE chain no longer
                    # bounds the period at (MM+ACT+DVE)/2 — the PE does.
                    psA = ppoolA.tile([128, half], F32, tag="psA")
                    psB = ppoolB.tile([128, half], F32, tag="psB")
                    for kp in range(kp_n):
                        for j in range(nsub):
                            if m == 0:
                                lhsT3 = atpa3[kp][:, :, :]
                            else:
                                lhsT3 = at3[kp][:, :, m * 128 : (m + 1) * 128]
                            if s == 0:
                                rhs3 = bt03[kp][j][:, :, :]
                            else:
                                rhs3 = bt3[s - 1][kp][:, :, j * 512 : (j + 1) * 512]
                            if j < nsub // 2:
                                dst = psA[:, j * 512 : (j + 1) * 512]
                            else:
                                dst = psB[:, (j - nsub // 2) * 512 : (j - nsub // 2 + 1) * 512]
                            nc.tensor.matmul(
                                dst,
                                lhsT3,
                                rhs3,
                                start=(kp == 0),
                                stop=(kp == kp_n - 1),
                                perf_mode=DR,
                            )
                    # ACT converts the front half of PSUM to fp16 SBUF; the
                    # single DVE op pairs each back-half PSUM element with its
                    # copied mate (TT may read at most ONE input from PSUM).
                    ca = apool.tile([128, half], F16, tag="ca")
                    nc.scalar.copy(out=ca, in_=psA)
                    l1 = lpool.tile([128, half], F16, tag="l1")
                    nc.vector.tensor_max(l1, psB, ca)
                    # Output DMAs alternate between the gpsimd and sync
                    # queues — one queue alone (~135 GB/s) would need
                    # ~1.9us/chunk for the 256KB l1 tile, above the PE
                    # period.
                    oeng = nc.gpsimd if (s * mt + m) % 2 == 0 else nc.sync
                    oeng.dma_start(
                        out=out_l1[m * 128 : (m + 1) * 128, s * half : (s + 1) * half],
                        in_=l1,
                    )
    nc.compile()
    return nc


def make_in_maps(a, b):
    """Pack per-core inputs.  Columns of each 2048-wide chunk are permuted so
    that the two level-1 reduction mates of device pair p hold the columns
    with c-sorted ranks 2p and 2p+1.  Device pairing: l1[p] = max of
    positions (p, 1024+p) [position p rides the ACT fp16 copy, 1024+p is
    read raw from PSUM], so position q maps to rank 2q for q<1024 and
    2(q-1024)+1 for q>=1024.
    Returns (in_maps, ranks) where ranks[core][s][r] is the local column with
    the r-th smallest c."""
    import ml_dtypes

    kt = D // 128
    kp_n = kt // 2
    aT8 = (2.0 * a).T.astype(ml_dtypes.float8_e4m3)   # [512, NA]
    bT8 = b.T.astype(ml_dtypes.float8_e4m3)           # [512, NB]
    b2 = np.einsum("ij,ij->i", b, b)
    sb = b.sum(axis=1)
    c = (b2 - np.float32(2.0 * EPS) * sb).astype(np.float32)
    nseg = NB_SHARD // CHUNK
    half = CHUNK // 2

    # device position q -> c-sorted rank: pair p = (p, 1024+p) -> (2p, 2p+1)
    q = np.arange(CHUNK)
    r_of_q = np.where(q < half, 2 * q, 2 * (q - half) + 1)

    def pair_pack(mat, kp):
        # [128, 2*cols]: slot i holds K-tile (kp*2+i) rows of mat
        lo = mat[(kp * 2) * 128 : (kp * 2 + 1) * 128, :]
        hi = mat[(kp * 2 + 1) * 128 : (kp * 2 + 2) * 128, :]
        return np.ascontiguousarray(np.concatenate([lo, hi], axis=1))

    in_maps = []
    all_ranks = []
    for core in range(NCORES):
        sl = slice(core * NB_SHARD, (core + 1) * NB_SHARD)
        bT = bT8[:, sl]
        c_core = c[core * NB_SHARD : (core + 1) * NB_SHARD]
        ranks = []
        im = {}
        for kp in range(kp_n):
            im[f"atp{kp}"] = pair_pack(aT8, kp)
            im[f"atp{kp}a"] = pair_pack(aT8[:, 0:128], kp)
        for g in range(nseg):
            cch = c_core[g * CHUNK : (g + 1) * CHUNK]
            rank = np.argsort(cch, kind="stable")
            ranks.append(rank)
            perm = rank[r_of_q]
            cols = np.ascontiguousarray(bT[:, g * CHUNK : (g + 1) * CHUNK][:, perm])
            if g == 0:
                for kp in range(kp_n):
                    for jj in range(CHUNK // 512):
                        im[f"bt0p{kp}j{jj}"] = pair_pack(
                            np.ascontiguousarray(cols[:, jj * 512 : (jj + 1) * 512]),
                            kp,
                        )
            else:
                for kp in range(kp_n):
                    im[f"bt{g}p{kp}"] = pair_pack(cols, kp)
        in_maps.append(im)
        all_ranks.append(ranks)
    return in_maps, all_ranks


def merge_results(a, b, n, b_batch_size, results, all_ranks):
    """Subtract the per-pair mean-c bias from the device pair-maxima, take
    the top-K pairs per (core, seg, query), expand each winner to its 2
    c-adjacent columns, refine with the exact fp32 reference distance, pick
    final top-n (ties -> lowest index), apply the reference's buggy
    bookkeeping.  Device pair p of (core, seg) covers the columns with
    c-ranks 2p and 2p+1 by construction of r_of_q."""
    nseg = NB_SHARD // CHUNK
    half = CHUNK // 2
    na = a.shape[0]
    b2 = np.sum(b * b, axis=1)
    sb_ = np.sum(b, axis=1)
    c = (b2 - np.float32(2.0 * EPS) * sb_).astype(np.float32)

    K = TOPK_PAIRS
    cand_parts = []
    for core in range(NCORES):
        l1 = results[core]["out_l1"].astype(np.float32)  # [NA, nseg*half]
        base_core = core * NB_SHARD
        for s in range(nseg):
            rank = all_ranks[core][s]
            # columns of pair p: ranks 2p, 2p+1
            cols0 = rank[0::2]  # [half]
            cols1 = rank[1::2]
            cpair = 0.5 * (c[base_core + s * CHUNK + cols0]
                           + c[base_core + s * CHUNK + cols1])
            corr = l1[:, s * half : (s + 1) * half] - cpair[None, :]
            top = np.argpartition(-corr, K, axis=1)[:, :K]  # [NA, K]
            base = base_core + s * CHUNK
            cand_parts.append(cols0[top] + base)
            cand_parts.append(cols1[top] + base)
    cand = np.concatenate(cand_parts, axis=1)  # [NA, NCORES*nseg*2K]

    a2 = np.sum(a * a, axis=1)
    sa = np.sum(a, axis=1)
    d = a.shape[1]
    out = np.empty((na, n), dtype=np.int64)
    CHQ = 128
    eps = np.float32(EPS)
    for q0 in range(0, na, CHQ):
        q1 = min(q0 + CHQ, na)
        Cc = cand[q0:q1]
        Bc = b[Cc]
        cross = np.matmul(Bc, a[q0:q1, :, None])[..., 0].astype(np.float32)
        sq = (
            a2[q0:q1, None]
            + b2[Cc]
            - np.float32(2.0) * cross
            + np.float32(2.0) * eps * (sa[q0:q1, None] - sb_[Cc])
            + np.float32(d) * eps * eps
        )
        dist = np.sqrt(np.maximum(sq, np.float32(0.0)))
        ordr = np.lexsort((Cc, dist), axis=1)[:, :n]
        rows = np.arange(q1 - q0)[:, None]
        out[q0:q1] = Cc[rows, ordr]
    buggy = (out % b_batch_size) + (out // b_batch_size)
    return buggy.astype(np.int64)


def kernel(a, b, n, b_batch_size, trace=False):
    from concourse.bass_utils import run_bass_kernel_spmd

    a = np.ascontiguousarray(np.asarray(a, dtype=np.float32))
    b = np.ascontiguousarray(np.asarray(b, dtype=np.float32))
    n = int(n)
    b_batch_size = int(b_batch_size)

    nc = build_kernel()
    in_maps, all_ranks = make_in_maps(a, b)
    res = run_bass_kernel_spmd(
        nc, in_maps, core_ids=list(range(NCORES)), trace=trace
    )
    out = merge_results(a, b, n, b_batch_size, res.results, all_ranks)
    if trace:
        return out, res
    return out
